# revision 3
# baseline (speedup 1.0000x reference)
"""Trainium2 Bass kernel for multi-head attention (B=8,S=1024,D=768,H=12).

Sharding: pure data-parallel over batch (B=8 == n_cores=8). Each core runs
the full attention for one batch element; no collectives needed.

Per-core pipeline (all matmuls fp32r, moving-dim 512):
  1. PE-transpose q,k,v into [D,S] layout.
  2. Projections: qhT/khT = (Wq^T q^T) in [D,S]; vh = v@Wv in [S,D].
     Biases folded via K=1 matmul rows (weights passed augmented [D+1,D]).
  3. Per head: scores psum = I@bias_nat + qhT^T@khT  (bias = adjoin + mask).
     ACT exp with fused row-sum accumulation, DVE reciprocal+scale -> attn.
  4. PE-transpose attn -> A^T tiles; AV: O^T = sum_i vh_i^T @ A^T_i (N=512).
  5. Output projection in natural layout from concatT, bias via ones-row.
"""

import sys

for _p in ("/opt/trn_rl_repo",):
    if _p not in sys.path:
        sys.path.insert(0, _p)

import numpy as np
from contextlib import ExitStack

import concourse.bass as bass
import concourse.tile as tile
from concourse import bacc, mybir
from concourse.bass_utils import run_bass_kernel_spmd

FP = mybir.dt.float32
FPR = mybir.dt.float32r
AF = mybir.ActivationFunctionType

B, S, D, H = 8, 1024, 768, 12
DEPTH = D // H  # 64
P = 128
NS = S // P  # 8 sequence tiles
ND = D // P  # 6 feature tiles

_NC_CACHE = {}


def _build_nc():
    nc = bacc.Bacc()

    q_d = nc.declare_dram_parameter("q", [S, D], FP, isOutput=False)
    k_d = nc.declare_dram_parameter("k", [S, D], FP, isOutput=False)
    v_d = nc.declare_dram_parameter("v", [S, D], FP, isOutput=False)
    bias_d = nc.declare_dram_parameter("bias", [S, S], FP, isOutput=False)
    ident_d = nc.declare_dram_parameter("ident", [P, P], FP, isOutput=False)
    ones_d = nc.declare_dram_parameter("ones", [1, S], FP, isOutput=False)
    wq_d = nc.declare_dram_parameter("wq", [D + 1, D], FP, isOutput=False)
    wk_d = nc.declare_dram_parameter("wk", [D + 1, D], FP, isOutput=False)
    wv_d = nc.declare_dram_parameter("wv", [D + 1, D], FP, isOutput=False)
    wo_d = nc.declare_dram_parameter("wo", [D + 1, D], FP, isOutput=False)
    out_d = nc.declare_dram_parameter("out", [S, D], FP, isOutput=True)
    attn_d = nc.declare_dram_parameter("attn", [H, S, S], FP, isOutput=True)

    with tile.TileContext(nc) as tc, ExitStack() as ctx:
        const = ctx.enter_context(tc.tile_pool(name="const", bufs=1))
        persist = ctx.enter_context(tc.tile_pool(name="persist", bufs=1))

        id_r = const.tile([P, P], FPR)
        nc.sync.dma_start(out=id_r, in_=ident_d[:, :].bitcast(FPR))
        ones_r = const.tile([1, S], FPR)
        nc.sync.dma_start(out=ones_r, in_=ones_d[:, :].bitcast(FPR))

        # persistent activations
        qhT = persist.tile([P, ND, S], FPR)  # 24KB/part
        khT = persist.tile([P, ND, S], FPR)  # 24KB/part
        vh = persist.tile([P, NS, D], FPR)  # 24KB/part
        bias_sb = persist.tile([P, NS, S], FPR)  # 32KB/part
        concatT = persist.tile([P, ND, S], FPR)  # 24KB/part

        # load bias tiles early (overlaps with projections)
        for st in range(NS):
            nc.sync.dma_start(
                out=bias_sb[:, st, :],
                in_=bias_d[st * P:(st + 1) * P, :].bitcast(FPR),
            )

        # ---------------- Phase A: transposes + projections ----------------
        with (
            tc.tile_pool(name="wpool", bufs=1) as wpool,
            tc.tile_pool(name="xnat", bufs=8) as xnat_pool,
            tc.tile_pool(name="xT", bufs=1) as xT_pool,
            tc.tile_pool(name="pst", bufs=2, space="PSUM") as pst_pool,
            tc.tile_pool(name="psp", bufs=2, space="PSUM") as psp_pool,
        ):
            for ti, (x_d, w_d, dst) in enumerate(
                ((q_d, wq_d, qhT), (k_d, wk_d, khT), (v_d, wv_d, vh))
            ):
                # weights for this projection
                w_sb = wpool.tile([P, ND, D], FPR, tag="w")
                for kc in range(ND):
                    nc.sync.dma_start(
                        out=w_sb[:, kc, :],
                        in_=w_d[kc * P:(kc + 1) * P, :].bitcast(FPR),
                    )
                wb_sb = wpool.tile([1, D], FPR, tag="wb")
                nc.sync.dma_start(out=wb_sb, in_=w_d[D:D + 1, :].bitcast(FPR))

                # load natural tiles
                xnat = []
                for st in range(NS):
                    t = xnat_pool.tile([P, D], FPR, tag="xn")
                    nc.sync.dma_start(
                        out=t, in_=x_d[st * P:(st + 1) * P, :].bitcast(FPR)
                    )
                    xnat.append(t)

                # transpose to xT [P, ND, S]
                xT = xT_pool.tile([P, ND, S], FPR, tag="xT")
                for dblk in range(ND):
                    for half in range(2):
                        pt = pst_pool.tile([P, 512], FPR, tag="pt")
                        for j in range(4):
                            st = half * 4 + j
                            nc.tensor.transpose(
                                pt[:, j * P:(j + 1) * P],
                                xnat[st][:, dblk * P:(dblk + 1) * P],
                                id_r,
                            )
                        eng = nc.vector if (dblk + half) % 2 == 0 else nc.scalar
                        if eng is nc.vector:
                            nc.vector.tensor_copy(
                                out=xT[:, dblk, half * 512:(half + 1) * 512],
                                in_=pt,
                            )
                        else:
                            nc.scalar.activation(
                                out=xT[:, dblk, half * 512:(half + 1) * 512],
                                in_=pt,
                                func=AF.Copy,
                            )

                if dst is vh:
                    # vh natural: for each s-tile: psum[s,dout] = sum_kc
                    #   vT[kc, s-block]^T @ wv[kc] + ones^T @ bias_row
                    for st in range(NS):
                        pp = psp_pool.tile([P, D], FP, tag="pp")
                        for nh, (n0, n1) in enumerate(((0, 512), (512, 768))):
                            for kc in range(ND):
                                nc.tensor.matmul(
                                    pp[:, n0:n1],
                                    xT[:, kc, st * P:(st + 1) * P],
                                    w_sb[:, kc, n0:n1],
                                    start=(kc == 0),
                                    stop=False,
                                )
                            nc.tensor.matmul(
                                pp[:, n0:n1],
                                ones_r[0:1, 0:P],
                                wb_sb[0:1, n0:n1],
                                start=False,
                                stop=True,
                            )
                        if st % 2 == 0:
                            nc.vector.tensor_copy(out=dst[:, st, :], in_=pp)
                        else:
                            nc.scalar.activation(out=dst[:, st, :], in_=pp, func=AF.Copy)
                else:
                    # qhT/khT: for each dout-tile: psum[dout, S] = sum_kc
                    #   wq[kc, dout-block]^T @ qT[kc] + bias_row^T @ ones
                    for dt_ in range(ND):
                        pp = psp_pool.tile([P, S], FP, tag="pp")
                        for half in range(2):
                            n0, n1 = half * 512, (half + 1) * 512
                            for kc in range(ND):
                                nc.tensor.matmul(
                                    pp[:, n0:n1],
                                    w_sb[:, kc, dt_ * P:(dt_ + 1) * P],
                                    xT[:, kc, n0:n1],
                                    start=(kc == 0),
                                    stop=False,
                                )
                            nc.tensor.matmul(
                                pp[:, n0:n1],
                                wb_sb[0:1, dt_ * P:(dt_ + 1) * P],
                                ones_r[0:1, n0:n1],
                                start=False,
                                stop=True,
                            )
                        if dt_ % 2 == 0:
                            nc.vector.tensor_copy(out=dst[:, dt_, :], in_=pp)
                        else:
                            nc.scalar.activation(out=dst[:, dt_, :], in_=pp, func=AF.Copy)

        # ---------------- Phase B: attention per head ----------------
        with (
            tc.tile_pool(name="psl", bufs=2, space="PSUM") as psl_pool,
            tc.tile_pool(name="pstr", bufs=2, space="PSUM") as pstr_pool,
            tc.tile_pool(name="pso", bufs=1, space="PSUM") as pso_pool,
            tc.tile_pool(name="expl", bufs=2) as expl_pool,
            tc.tile_pool(name="attn", bufs=11) as attn_pool,
            tc.tile_pool(name="at", bufs=2) as at_pool,
            tc.tile_pool(name="small", bufs=4) as small_pool,
        ):
            for h in range(H):
                t_h = h // 2
                r0 = (h % 2) * DEPTH
                attn_tiles = []
                for st in range(NS):
                    psl = psl_pool.tile([P, S], FP, tag="psl")
                    for half in range(2):
                        n0, n1 = half * 512, (half + 1) * 512
                        nc.tensor.matmul(
                            psl[:, n0:n1],
                            id_r,
                            bias_sb[:, st, n0:n1],
                            start=True,
                            stop=False,
                        )
                        nc.tensor.matmul(
                            psl[:, n0:n1],
                            qhT[r0:r0 + DEPTH, t_h, st * P:(st + 1) * P],
                            khT[r0:r0 + DEPTH, t_h, n0:n1],
                            start=False,
                            stop=True,
                        )
                    expl = expl_pool.tile([P, S], FP, tag="expl")
                    rowsum = small_pool.tile([P, 1], FP, tag="rs")
                    nc.scalar.activation(
                        out=expl, in_=psl, func=AF.Exp, accum_out=rowsum
                    )
                    recip = small_pool.tile([P, 1], FP, tag="rc")
                    nc.vector.reciprocal(out=recip, in_=rowsum)
                    at_t = attn_pool.tile([P, S], FPR, tag="attn")
                    nc.vector.tensor_scalar_mul(at_t, expl.bitcast(FPR), recip)
                    nc.sync.dma_start(
                        out=attn_d[h, st * P:(st + 1) * P, :].bitcast(FPR),
                        in_=at_t,
                    )
                    attn_tiles.append(at_t)

                # transpose attn -> A^T, AV accumulate
                pso = pso_pool.tile([DEPTH, S], FP, tag="pso")
                for i in range(NS):
                    at_sb = at_pool.tile([P, S], FPR, tag="at")
                    for half in range(2):
                        pt = pstr_pool.tile([P, 512], FPR, tag="ptr")
                        for j in range(4):
                            st = half * 4 + j
                            nc.tensor.transpose(
                                pt[:, j * P:(j + 1) * P],
                                attn_tiles[st][:, i * P:(i + 1) * P],
                                id_r,
                            )
                        if (i + half) % 2 == 0:
                            nc.vector.tensor_copy(
                                out=at_sb[:, half * 512:(half + 1) * 512], in_=pt
                            )
                        else:
                            nc.scalar.activation(
                                out=at_sb[:, half * 512:(half + 1) * 512],
                                in_=pt,
                                func=AF.Copy,
                            )
                    for half in range(2):
                        n0, n1 = half * 512, (half + 1) * 512
                        nc.tensor.matmul(
                            pso[:, n0:n1],
                            vh[:, i, h * DEPTH:(h + 1) * DEPTH],
                            at_sb[:, n0:n1],
                            start=(i == 0),
                            stop=(i == NS - 1),
                        )
                nc.vector.tensor_copy(out=concatT[r0:r0 + DEPTH, t_h, :], in_=pso)

        # ---------------- Phase C: output projection ----------------
        with (
            tc.tile_pool(name="wo", bufs=1) as wo_pool,
            tc.tile_pool(name="osb", bufs=3) as out_pool,
            tc.tile_pool(name="psq", bufs=2, space="PSUM") as psq_pool,
        ):
            wo_sb = wo_pool.tile([P, ND, D], FPR, tag="wo")
            for kc in range(ND):
                nc.sync.dma_start(
                    out=wo_sb[:, kc, :], in_=wo_d[kc * P:(kc + 1) * P, :].bitcast(FPR)
                )
            wob_sb = wo_pool.tile([1, D], FPR, tag="wob")
            nc.sync.dma_start(out=wob_sb, in_=wo_d[D:D + 1, :].bitcast(FPR))

            for st in range(NS):
                pq = psq_pool.tile([P, D], FP, tag="pq")
                for n0, n1 in ((0, 512), (512, 768)):
                    for kc in range(ND):
                        nc.tensor.matmul(
                            pq[:, n0:n1],
                            concatT[:, kc, st * P:(st + 1) * P],
                            wo_sb[:, kc, n0:n1],
                            start=(kc == 0),
                            stop=False,
                        )
                    nc.tensor.matmul(
                        pq[:, n0:n1],
                        ones_r[0:1, 0:P],
                        wob_sb[0:1, n0:n1],
                        start=False,
                        stop=True,
                    )
                o_sb = out_pool.tile([P, D], FP, tag="osb")
                if st % 2 == 0:
                    nc.vector.tensor_copy(out=o_sb, in_=pq)
                else:
                    nc.scalar.activation(out=o_sb, in_=pq, func=AF.Copy)
                nc.sync.dma_start(out=out_d[st * P:(st + 1) * P, :], in_=o_sb)

    nc.finalize()
    return nc


def _prep_in_maps(v, k, q, mask, adjoin_matrix, wq, bq, wk, bk, wv, bv, wo, bo):
    v = np.asarray(v, dtype=np.float32)
    k = np.asarray(k, dtype=np.float32)
    q = np.asarray(q, dtype=np.float32)
    mask = np.asarray(mask, dtype=np.float32)
    adjoin_matrix = np.asarray(adjoin_matrix, dtype=np.float32)
    wq = np.asarray(wq, dtype=np.float32)
    bq = np.asarray(bq, dtype=np.float32)
    wk = np.asarray(wk, dtype=np.float32)
    bk = np.asarray(bk, dtype=np.float32)
    wv = np.asarray(wv, dtype=np.float32)
    bv = np.asarray(bv, dtype=np.float32)
    wo = np.asarray(wo, dtype=np.float32)
    bo = np.asarray(bo, dtype=np.float32)

    scale = 1.0 / np.sqrt(np.float32(DEPTH))
    wq_aug = np.concatenate([wq * scale, (bq * scale)[None, :]], axis=0)
    wk_aug = np.concatenate([wk, bk[None, :]], axis=0)
    wv_aug = np.concatenate([wv, bv[None, :]], axis=0)
    wo_aug = np.concatenate([wo, bo[None, :]], axis=0)
    # bias[sq, sk] = adjoin[b,0,sq,sk] + (-1e9)*mask[b,0,0,sk]
    bias = adjoin_matrix[:, 0, :, :] + (-1e9) * mask[:, 0, 0, :][:, None, :]
    ident = np.eye(P, dtype=np.float32)
    ones = np.ones((1, S), dtype=np.float32)

    in_maps = [
        {
            "q": np.ascontiguousarray(q[b]),
            "k": np.ascontiguousarray(k[b]),
            "v": np.ascontiguousarray(v[b]),
            "bias": np.ascontiguousarray(bias[b]),
            "ident": ident,
            "ones": ones,
            "wq": wq_aug,
            "wk": wk_aug,
            "wv": wv_aug,
            "wo": wo_aug,
        }
        for b in range(B)
    ]
    return in_maps


def kernel(**inputs):
    if "nc" not in _NC_CACHE:
        _NC_CACHE["nc"] = _build_nc()
    nc = _NC_CACHE["nc"]
    in_maps = _prep_in_maps(**inputs)
    res = run_bass_kernel_spmd(nc, in_maps, list(range(B))).results
    output = np.stack([res[b]["out"] for b in range(B)], axis=0)
    attn = np.stack([res[b]["attn"] for b in range(B)], axis=0)
    return output, attn
